# revision 57
# baseline (speedup 1.0000x reference)
"""CIN (Compressed Interaction Network) kernel for Trainium2, SPMD over 8 cores.

Reference computation (per layer l, with x0 = embeddings (B, M, D)):
    xk = relu(einsum("bmd,bhd,mhk->bkd", x0, x_{k-1}, W_l))   # (B, K, D)
    out_l = sum_d xk                                           # (B, K)
Output: concat(out_0, out_1, out_2) -> (B, 192)

Mapping (per core, B_loc = 2048 batch rows, data-parallel over B):
  * Interaction index (m,h) / output index k live on SBUF partitions;
    (b, d) is flattened on the free dim (N = BT*D per b-tile).
  * For each 128-row block g of the (m,h) interaction space:
      bc_g   = x0[m(p)] broadcast per partition  (DMA from DRAM with a
               zero-stride access pattern -- no compute engine involved)
      v_g    = bc_g * XkRep       (bf16 tensor_tensor, DVE / GPSIMD)
      out   += Wdup_g.T @ v_g     (PE, fp32 PSUM accumulation)
  * Wdup_g = [W_g | W_g] (host-duplicated) so the PSUM result lands
    duplicated in both partition halves -> after ReLU the SBUF tile is
    directly the next layer's replicated Xk (XkRep[p] = xk[p % 64]).
  * All 16 bc64 blocks for a b-tile live in ONE [128,16,N] tile loaded by
    2 DMAs; the 8 bc32 blocks in ONE [128,8,N] tile by 4 DMAs (the SP
    sequencer's per-DMA issue cost is the kernel's top bottleneck, so DMA
    *count* matters more than DMA bytes). bc DMAs issue from the mostly
    idle Activation queue, xt/out from SP.
  * ReLU via ScalarE PSUM->SBUF (bf16); per-layer d-sums via DVE reduce;
    outputs stored k-major (192, B_loc) one DMA per tile-pair,
    transposed/concatenated on host.

All matmul/TT data is bf16 (fp32 PSUM accumulation); norm rel err ~2e-3.

Self-contained: hardcodes shapes from the problem spec.
"""

import os

import ml_dtypes
import numpy as np

B, M, D = 16384, 32, 16
N_CORES = 8
B_LOC = B // N_CORES

BT = 64  # batch rows per b-tile
N_FREE = BT * D  # 1024 free elems per b-tile
N_TILES = B_LOC // BT
MM_FREE = 512  # one fp32 PSUM bank; max free dim per matmul

N_TILES_BUILD = int(os.environ.get("CIN_NTILES", str(N_TILES)))
DEV_REPS = int(os.environ.get("CIN_DEVREPS", "1"))  # on-device repeat loop (timing only)
ABL_NO_TT = int(os.environ.get("CIN_ABL_NO_TT", "0"))    # matmul reads bc directly
ABL_NO_MM = int(os.environ.get("CIN_ABL_NO_MM", "0"))    # skip matmuls+relu+reduce
ABL_NO_BC = int(os.environ.get("CIN_ABL_NO_BC", "0"))    # TT reads xt instead of bc (skip bc DMAs)
ABL_BC_HALF = int(os.environ.get("CIN_ABL_BC_HALF", "0"))  # timing-only: halve bc64 DMA
PB_N = int(os.environ.get("CIN_PB_N", "0"))  # last n bc64 blocks via gpsimd partition_broadcast
PEBC_N = int(os.environ.get("CIN_PEBC_N", "16"))  # bc64 blocks via PE sel-matmul + copy
PEBC_POOL = int(os.environ.get("CIN_PEBC_POOL", "0"))  # of those, n copies done by gpsimd
PEBC32 = int(os.environ.get("CIN_PEBC32", "1"))  # bc32 on-chip too (PE + Act/DVE copies)
# every GPSIMD_EVERY-th TT batch goes to GPSIMD (1000 = never)
GPSIMD_EVERY = int(os.environ.get("CIN_GPSIMD_EVERY", "1000"))
JB = int(os.environ.get("CIN_JB", "1"))  # interaction blocks per TT instruction
VBUFS = int(os.environ.get("CIN_VBUFS", "12"))
RED_DELAY = int(os.environ.get("CIN_RED_DELAY", "400"))
BC_EARLY = int(os.environ.get("CIN_BC_EARLY", "200"))
OUTPS_BUFS = int(os.environ.get("CIN_OUTPS", "2"))
BC64_BUFS = int(os.environ.get("CIN_BC64BUFS", "3"))
BC32_BUFS = int(os.environ.get("CIN_BC32BUFS", "3"))
XIN_BUFS = int(os.environ.get("CIN_XINBUFS", "3"))
BC_ENG = os.environ.get("CIN_BCENG", "sp")  # dedicated queue: bc DMAs only
XT_ENG = os.environ.get("CIN_XTENG", "sp")  # keep Act queue free of DMA issues
OUT_ENG = os.environ.get("CIN_OUTENG", "sp")
BC_SRC = os.environ.get("CIN_BCSRC", "x0t")  # x0t: 2KB strided descs; eo: contiguous
BC_SPLIT = int(os.environ.get("CIN_BCSPLIT", "0"))  # split bc DMAs across sp/act/pool queues
EOD = int(os.environ.get("CIN_EOD", "1"))  # eo-layout descriptor size, in N_FREE units
RED_ENG = os.environ.get("CIN_REDENG", "dve")  # d-sum reduce engine (free-axis: DVE only)

_CACHE = {}


def _prep_weights(W0, W1, W2):
    """Flatten (m,h)->rows, split into 128-row blocks, duplicate along k."""
    out = {}
    for i, W in enumerate((W0, W1, W2)):
        m, h, k = W.shape
        flat = np.ascontiguousarray(np.asarray(W, dtype=np.float32).reshape(m * h, k))
        G = (m * h) // 128
        blocks = flat.reshape(G, 128, k)
        dup = np.concatenate([blocks, blocks], axis=2)  # (G, 128, 128)
        out[f"w{i}dup"] = np.ascontiguousarray(dup.astype(ml_dtypes.bfloat16))
    if PEBC_N:
        # selection matrices: S[jj][c, p] = 1 iff c == 2*j + p//64 (row of xt
        # holding x0[2j+a]); PE matmul S^T @ xt reproduces bc64 block j in PSUM
        S = np.zeros((PEBC_N, 128, 128), dtype=np.float32)
        for jj, j in enumerate(range(16 - PEBC_N, 16)):
            for p in range(128):
                S[jj, 2 * j + p // 64, p] = 1.0
        out["selbc"] = np.ascontiguousarray(S.astype(ml_dtypes.bfloat16))
        if PEBC_POOL:
            out["ones1"] = np.ones((128, 1), dtype=ml_dtypes.bfloat16)
    if PEBC32:
        S2 = np.zeros((8, 128, 128), dtype=np.float32)
        for g in range(8):
            for p in range(128):
                S2[g, 4 * g + p // 32, p] = 1.0
        out["sel32"] = np.ascontiguousarray(S2.astype(ml_dtypes.bfloat16))
    return out


def _deinterleave(x0t):
    """Host-side copies of x0t giving per-(partition-group, tile) contiguous
    bc DMA source runs. x0eo[a,(t,jj,f)] = x0t[2jj+a, t*N+f]; x0q similarly
    for stride-4 row groups."""
    x3 = x0t.reshape(M, N_TILES, N_FREE)
    x0eo = np.ascontiguousarray(
        x3.reshape(16, 2, N_TILES, N_FREE).transpose(1, 2, 0, 3).reshape(2, -1)
    )
    x0q = np.ascontiguousarray(
        x3.reshape(8, 4, N_TILES, N_FREE).transpose(1, 2, 0, 3).reshape(4, -1)
    )
    return {"x0eo": x0eo, "x0q": x0q}


def _build_bass():
    import concourse.bass as bass
    import concourse.mybir as mybir
    import concourse.tile as tile
    from concourse import bacc

    f32 = mybir.dt.float32
    bf16 = mybir.dt.bfloat16

    nc = bacc.Bacc(None, target_bir_lowering=False, debug=False)

    # x0 transposed to (m, b*d), bf16
    x0t = nc.dram_tensor("x0t", (M, B_LOC * D), bf16, kind="ExternalInput")
    # host-deinterleaved copies of x0t: per-(partition-group, tile) contiguous
    # runs so each bc DMA descriptor covers a whole tile row (32KB / 8KB)
    # x0eo[a, t, jj, f] = x0t[2*jj+a, t*N_FREE+f]
    x0eo = nc.dram_tensor("x0eo", (2, N_TILES * 16 * N_FREE), bf16, kind="ExternalInput")
    # x0q[s, t, j, f] = x0t[4*j+s, t*N_FREE+f]
    x0q = nc.dram_tensor("x0q", (4, N_TILES * 8 * N_FREE), bf16, kind="ExternalInput")
    w_dram = [
        nc.dram_tensor("w0dup", (8, 128, 128), bf16, kind="ExternalInput"),
        nc.dram_tensor("w1dup", (16, 128, 128), bf16, kind="ExternalInput"),
        nc.dram_tensor("w2dup", (16, 128, 128), bf16, kind="ExternalInput"),
    ]
    out_dram = nc.dram_tensor("out", (192, B_LOC), f32, kind="ExternalOutput")

    ROW = B_LOC * D  # x0t row stride in elements

    with tile.TileContext(nc) as tc:
        with (
            tc.tile_pool(name="consts", bufs=1) as consts,
            tc.tile_pool(name="xin", bufs=XIN_BUFS) as xin,
            tc.tile_pool(name="bc64p", bufs=BC64_BUFS) as bc64p,
            tc.tile_pool(name="bc32p", bufs=BC32_BUFS) as bc32p,
            tc.tile_pool(name="xk", bufs=6) as xkp,
            tc.tile_pool(name="vbuf", bufs=VBUFS) as vbuf,
            tc.tile_pool(name="obuf", bufs=3) as obuf,
            tc.tile_pool(name="outps", bufs=OUTPS_BUFS, space="PSUM") as outps,
            tc.tile_pool(name="pebc", bufs=4, space="PSUM") as pebcp,
        ):
            eng = {"sp": nc.sync, "act": nc.scalar, "dve": nc.vector,
                   "pool": nc.gpsimd}
            bc_eng = eng[BC_ENG]
            xt_eng = eng[XT_ENG]

            w_sb = []
            for i, (wd, G) in enumerate(zip(w_dram, (8, 16, 16))):
                t = consts.tile([128, G, 128], bf16, tag=f"w{i}")
                nc.sync.dma_start(out=t, in_=wd.rearrange("g p q -> p g q"))
                w_sb.append(t)
            sel_sb = ones_sb = None
            if PEBC_N:
                sel_d = nc.dram_tensor("selbc", (PEBC_N, 128, 128), bf16,
                                       kind="ExternalInput")
                sel_sb = consts.tile([128, PEBC_N, 128], bf16, tag="selbc")
                nc.sync.dma_start(out=sel_sb, in_=sel_d.rearrange("j c p -> c j p"))
                if PEBC_POOL:
                    ones_d = nc.dram_tensor("ones1", (128, 1), bf16,
                                            kind="ExternalInput")
                    ones_sb = consts.tile([128, 1], bf16, tag="ones1")
                    nc.sync.dma_start(out=ones_sb, in_=ones_d[:, :])
            sel32_sb = None
            if PEBC32:
                sel32_d = nc.dram_tensor("sel32", (8, 128, 128), bf16,
                                         kind="ExternalInput")
                sel32_sb = consts.tile([128, 8, 128], bf16, tag="sel32")
                nc.sync.dma_start(out=sel32_sb, in_=sel32_d.rearrange("j c p -> c j p"))

            def load_tile(t_i, outs_all):
                off = t_i * N_FREE
                import contextlib
                bccm = (lambda: tc.high_priority(offset=BC_EARLY)) if BC_EARLY else contextlib.nullcontext
                xt = xin.tile([128, N_FREE], bf16, tag="x0x4")
                for s in range(4):
                    with bccm():
                        xt_eng.dma_start(
                            out=xt[32 * s : 32 * (s + 1), :],
                            in_=x0t[:, off : off + N_FREE],
                        )
                b32 = b64 = None
                gen = []
                n_gen = 0

                def gen_copy(dst, ps, idx):
                    # every 3rd generated block's copies go to DVE, rest to Act
                    if idx % 3 == 2:
                        nc.vector.tensor_scalar_add(dst, ps, 0.0)
                    else:
                        nc.scalar.activation(dst, ps, mybir.ActivationFunctionType.Copy)

                if not ABL_NO_BC:
                    b32 = bc32p.tile([128, 8, N_FREE], bf16, tag="bc32")
                    if PEBC32:
                        for g in range(8):
                            def gen_b32(g=g, b32=b32, xt=xt, idx=n_gen):
                                for h0 in range(0, N_FREE, MM_FREE):
                                    ps = pebcp.tile([128, MM_FREE], f32, tag="pebc")
                                    nc.tensor.matmul(
                                        ps, lhsT=sel32_sb[:, g, :],
                                        rhs=xt[:, h0 : h0 + MM_FREE],
                                        start=True, stop=True,
                                    )
                                    gen_copy(b32[:, g, h0 : h0 + MM_FREE], ps, idx)
                            gen.append(gen_b32)
                            n_gen += 1
                    for s in range(4 if not PEBC32 else 0):  # quarter s reads rows 4j+s
                        if BC_SRC == "eo":
                            eod = min(EOD, 8) * N_FREE
                            src = bass.AP(
                                tensor=x0q,
                                offset=(s * N_TILES + t_i) * 8 * N_FREE,
                                ap=[[0, 32], [eod, 8 * N_FREE // eod], [1, eod]],
                            )
                        else:
                            src = bass.AP(
                                tensor=x0t,
                                offset=s * ROW + off,
                                ap=[[0, 32], [4 * ROW, 8], [1, N_FREE]],
                            )
                        if BC_SPLIT == 2:
                            e = eng[("sp", "act", "pool", "pool")[s]]
                        elif BC_SPLIT:
                            e = eng["act" if s % 2 else BC_ENG]
                        else:
                            e = bc_eng
                        dst = b32[32 * s : 32 * (s + 1), :, :]
                        if BC_SRC == "eo" and EOD > 1:
                            eod = min(EOD, 8) * N_FREE
                            dst = dst.rearrange("p j f -> p (j f)").rearrange(
                                "p (c e) -> p c e", e=eod)
                        with bccm():
                            e.dma_start(out=dst, in_=src)
                    b64 = bc64p.tile([128, 16, N_FREE], bf16, tag="bc64")
                    n_dma_j = (8 if ABL_BC_HALF else 16) - PB_N - PEBC_N
                    for a in range(2 if n_dma_j > 0 else 0):  # half a reads rows 2j+a
                        dst = b64[64 * a : 64 * (a + 1), :n_dma_j, :]
                        if BC_SRC == "eo":
                            eod = min(EOD, n_dma_j) * N_FREE
                            src = bass.AP(
                                tensor=x0eo,
                                offset=(a * N_TILES + t_i) * 16 * N_FREE,
                                ap=[[0, 64], [eod, n_dma_j * N_FREE // eod], [1, eod]],
                            )
                            if EOD > 1:
                                dst = dst.rearrange("p j f -> p (j f)").rearrange(
                                    "p (c e) -> p c e", e=eod)
                        else:
                            src = bass.AP(
                                tensor=x0t,
                                offset=a * ROW + off,
                                ap=[[0, 64], [2 * ROW, n_dma_j], [1, N_FREE]],
                            )
                        e = eng["act" if (BC_SPLIT and a % 2) else BC_ENG]
                        with bccm():
                            e.dma_start(out=dst, in_=src)
                    for j in range(16 - PB_N, 16):  # on-chip broadcast via Pool
                        for a in range(2):
                            nc.gpsimd.partition_broadcast(
                                b64[64 * a : 64 * (a + 1), j, :],
                                xt[2 * j + a - 32 : 2 * j + a - 31, :],
                            )
                    for j in range(16 - PEBC_N, 16):  # on-chip: PE sel-matmul + copy
                        jj = j - (16 - PEBC_N)

                        def gen_block(j=j, jj=jj, b64=b64, xt=xt, idx=n_gen):
                            for h0 in range(0, N_FREE, MM_FREE):
                                ps = pebcp.tile([128, MM_FREE], f32, tag="pebc")
                                nc.tensor.matmul(
                                    ps,
                                    lhsT=sel_sb[:, jj, :],
                                    rhs=xt[:, h0 : h0 + MM_FREE],
                                    start=True,
                                    stop=True,
                                )
                                gen_copy(b64[:, j, h0 : h0 + MM_FREE], ps, idx)

                        gen.append(gen_block)
                        n_gen += 1
                return {"xt": xt, "b32": b32, "b64": b64, "t_i": t_i,
                        "xk_rep": xt, "pending": None, "outs": outs_all,
                        "slot": t_i % 2, "n_red": 0, "gen": gen}

            red_eng = {"dve": nc.vector, "pool": nc.gpsimd}[RED_ENG]

            def emit_reduce(st, layer, xk_tile):
                import contextlib
                cm = tc.high_priority(offset=-RED_DELAY) if RED_DELAY else contextlib.nullcontext()
                with cm:
                    red_eng.reduce_sum(
                        out=st["outs"][:, layer, st["slot"], :],
                        in_=xk_tile[:64].rearrange("k (b d) -> k b d", d=D),
                        axis=mybir.AxisListType.X,
                    )
                st["n_red"] += 1

            def emit_layer(st, layer):
                G = 8 if layer == 0 else 16
                W = w_sb[layer]
                bct = st["b32"] if layer == 0 else st["b64"]
                xk_rep = st["xk_rep"]
                ops = outps.tile([128, N_FREE], f32, tag="outps")
                nb = G // JB
                if layer == 0:
                    for _ in range(min(len(st["gen"]), 3)):  # prime gen lookahead
                        st["gen"].pop(0)()
                for jb in range(nb):
                    if ABL_NO_TT or ABL_NO_BC:
                        v = None  # matmul reads bc (or xt) directly per block
                    else:
                        v = vbuf.tile([128, JB, N_FREE], bf16, tag="v")
                        j0 = jb * JB
                        if ABL_BC_HALF and layer > 0:
                            j0 = (jb % 8) * JB  # timing-only aliasing
                        in0 = bct[:, j0 : j0 + JB, :]
                        in1 = xk_rep.unsqueeze(1).broadcast_to([128, JB, N_FREE])
                        if jb % GPSIMD_EVERY == GPSIMD_EVERY - 1:
                            nc.gpsimd.tensor_mul(v, in0, in1)
                        else:
                            nc.vector.tensor_mul(v, in0, in1)
                    if not ABL_NO_MM:
                        for j in range(JB):
                            g = jb * JB + j
                            if v is None:
                                rsrc = st["xt"] if ABL_NO_BC else bct[:, g, :]
                            else:
                                rsrc = v[:, j, :]
                            for h0 in range(0, N_FREE, MM_FREE):
                                nc.tensor.matmul(
                                    ops[:, h0 : h0 + MM_FREE],
                                    lhsT=W[:, g, :],
                                    rhs=rsrc[:, h0 : h0 + MM_FREE],
                                    start=(g == 0),
                                    stop=(g == G - 1),
                                )
                    if st["pending"] is not None and jb == nb // 2:
                        emit_reduce(st, *st["pending"])
                        st["pending"] = None
                    if layer <= 1 and st["gen"]:
                        # spread bc generation through L0/L1 so the PE sees it
                        # interleaved, not bunched at tile start
                        st["gen"].pop(0)()
                if ABL_NO_MM:
                    st["xk_rep"] = st["xt"]
                    if st["pending"] is None:
                        st["pending"] = (layer, st["xt"])
                    return
                xk_new = xkp.tile([128, N_FREE], bf16, tag="xk")
                nc.scalar.activation(xk_new, ops, mybir.ActivationFunctionType.Relu)
                st["pending"] = (layer, xk_new)
                st["xk_rep"] = xk_new

            def whole_pass():
                for pair in range(N_TILES_BUILD // 2):
                    outs_all = obuf.tile([64, 3, 2, BT], f32, tag="outs")
                    stA = load_tile(2 * pair, outs_all)
                    stB = load_tile(2 * pair + 1, outs_all)
                    for layer in range(3):
                        emit_layer(stA, layer)
                        emit_layer(stB, layer)
                    emit_reduce(stA, *stA["pending"])
                    emit_reduce(stB, *stB["pending"])
                    # one store per pair: (64k, 3layer, 2tiles*64b)
                    eng[OUT_ENG].dma_start(
                        out=bass.AP(
                            tensor=out_dram,
                            offset=pair * 2 * BT,
                            ap=[[B_LOC, 64], [64 * B_LOC, 3], [1, 2 * BT]],
                        ),
                        in_=outs_all.rearrange("k l s b -> k l (s b)"),
                    )

            if DEV_REPS > 1:
                with tc.For_i(0, DEV_REPS, 1):
                    whole_pass()
            else:
                whole_pass()

    nc.finalize()
    return nc


def _get_program():
    if "nc" not in _CACHE:
        _CACHE["nc"] = _build_bass()
    return _CACHE["nc"]


def kernel(embeddings, W0, W1, W2):
    from concourse.bass_utils import run_bass_kernel_spmd

    embeddings = np.asarray(embeddings, dtype=np.float32)
    wmaps = _prep_weights(np.asarray(W0), np.asarray(W1), np.asarray(W2))

    in_maps = []
    for c in range(N_CORES):
        emb = embeddings[c * B_LOC : (c + 1) * B_LOC]  # (B_LOC, M, D)
        x0t = np.ascontiguousarray(
            emb.transpose(1, 0, 2).reshape(M, B_LOC * D).astype(ml_dtypes.bfloat16)
        )
        in_maps.append({"x0t": x0t, **_deinterleave(x0t), **wmaps})

    nc = _get_program()
    kwargs = {}
    if os.environ.get("CIN_TRACE", "0") == "1":
        kwargs["trace"] = True
        tmpdir = os.environ.get("CIN_TRACE_DIR")
        if tmpdir:
            kwargs["tmpdir"] = tmpdir
    res = run_bass_kernel_spmd(nc, in_maps, core_ids=list(range(N_CORES)), **kwargs)
    if res.exec_time_ns is not None:
        _CACHE["exec_time_ns"] = res.exec_time_ns
        _CACHE["trace"] = res.instructions_and_trace
        _CACHE["profile_json"] = res.profile_json

    outs = [r["out"].T for r in res.results]  # each (B_LOC, 192)
    return np.ascontiguousarray(np.concatenate(outs, axis=0))
